# revision 3
# baseline (speedup 1.0000x reference)
"""Trainium2 Bass/Tile kernel for masked multi-head attention.

Reference computation (per batch b):
  q = leaky(X_q @ WQ.T + bQ); k = leaky(X_k @ WK.T + bK); v = leaky(X_v @ WV.T + bV)
  scores_h = (q_h @ k_h.T + NEG*(1 - qm x km)) / 8
  attn = softmax_k(scores) * qm;  out_h = attn_h @ v_h

Sharding: data-parallel over batch, 2 batches per core on 8 cores.

End-to-end wall time is dominated by the axon tunnel (~75 MB/s half-duplex),
so the dispatch path minimizes wire bytes:
  - rows with qm==0 produce exactly-zero output rows, and rows with km==0
    contribute exactly zero to every softmax (additive -2^32 mask -> exp==0),
    so the host sends only mask-selected rows, padded to SP (multiple of 128).
    This halves q/k/v upload AND output download, and shrinks attention
    compute by (SP/S)^2.
  - all wire tensors are bf16; output is upcast host-side
  - weights are sent pre-transposed as 1/8 row-shards and AllGathered on
    device over NeuronLink instead of 8x-replicated over the tunnel
  - donated zero output buffers are created on device (never transferred)
  - host packing pipelines against async device_put transfers
  - the shard_map'd bass_exec call is AOT-compiled once and cached per SP

Per-core dataflow (all matmuls bf16 operands, fp32 PSUM accumulation):
  - X loaded natural, PE-transposed to XT [d, s] (d on partitions).
  - qT/kT computed transposed [d', s]; v computed natural [s, d'].
  - Masking: exp((s + mask)/8) == exp(s/8)*qm[q]*km[k]; km is folded into an
    augmented V: v_aug = [leaky(v)*km | km], so the AV matmul produces the
    masked numerator and the softmax denominator (last column).  qm is applied
    in the final normalization.  No row-max subtraction: |scores/8| < ~6.
  - scoresT[k, q] = kT_h.T @ qT_h per 128-k-chunk, exp on ACT straight out of
    PSUM, AV accumulates outT[65, q] = v_aug.T @ exp_scoresT over k-chunks.
  - outT is PE-transposed back to [q, d'] and normalized with recip(denom)*qm.
"""

import numpy as np
import ml_dtypes
from contextlib import ExitStack

import jax
import jax.numpy as jnp
from jax.sharding import Mesh, PartitionSpec, NamedSharding

import concourse.bass as bass
import concourse.tile as tile
from concourse import bacc, mybir
from concourse.bass2jax import (
    _bass_exec_p,
    partition_id_tensor,
    install_neuronx_cc_hook,
)
from concourse.masks import make_identity

B, S, D, H = 16, 1024, 512, 8
DH = D // H          # 64
NCORES = 8
BL = B // NCORES     # batches per core
DC = D // 128        # 4 d-chunks

F32 = mybir.dt.float32
BF16 = mybir.dt.bfloat16
F8E3 = mybir.dt.float8e3
FP16 = mybir.dt.float16
AF = mybir.ActivationFunctionType
ALU = mybir.AluOpType

NP_BF16 = ml_dtypes.bfloat16
NP_F8E3 = ml_dtypes.float8_e3m4
NP_F16 = np.float16

_LUT = None


def _f16_to_e3m4_lut():
    # double-rounds f32 -> f16 -> e3m4; ~3x faster than ml_dtypes' scalar
    # cast and at most 1 ulp off on round-to-even ties
    global _LUT
    if _LUT is None:
        with np.errstate(invalid="ignore", over="ignore"):
            _LUT = (np.arange(65536, dtype=np.uint16).view(np.float16)
                    .astype(NP_F8E3).view(np.uint8))
    return _LUT


def _col_chunks(n):
    """Split [0, n) into PE-matmul-sized column chunks (<=512 wide)."""
    out, s = [], 0
    while s < n:
        w = min(512, n - s)
        out.append((s, w))
        s += w
    return out


def _mha_body(ctx: ExitStack, tc: tile.TileContext, io: dict, use_bias: bool,
              SP: int, SQ: int, SK: int):
    nc = tc.nc
    SC = SP // 128
    PW = max(SP, 512)   # pa-pool tile width (v-proj needs 512 cols)
    chunks = _col_chunks(SP)
    ngrp = (SC + 3) // 4   # transpose-back groups of 4 q-chunks per 512 cols

    const = ctx.enter_context(tc.tile_pool(name="const", bufs=1))
    xstage = ctx.enter_context(tc.tile_pool(name="xstage", bufs=6))
    xtpool = ctx.enter_context(tc.tile_pool(name="xt", bufs=1))
    qkv = ctx.enter_context(tc.tile_pool(name="qkv", bufs=1))
    sepool = ctx.enter_context(tc.tile_pool(name="se", bufs=3))
    otpool = ctx.enter_context(tc.tile_pool(name="ot", bufs=2))
    smalls = ctx.enter_context(tc.tile_pool(name="smalls", bufs=2))
    outsp = ctx.enter_context(tc.tile_pool(name="outs", bufs=1))
    pa = ctx.enter_context(tc.tile_pool(name="pa", bufs=2, space="PSUM"))
    pb = ctx.enter_context(tc.tile_pool(name="pb", bufs=2, space="PSUM"))
    dram = ctx.enter_context(tc.tile_pool(name="dram", bufs=1, space="DRAM"))

    ident = const.tile([128, 128], F32, tag="ident")
    make_identity(nc, ident[:])
    identb = const.tile([128, 128], FP16, tag="identb")
    make_identity(nc, identb[:])

    def split_copy(dst, src, ncols):
        # drain a PSUM slot to SBUF in two DVE ops (pipelines against PE fill)
        h = ncols // 2
        nc.vector.tensor_copy(dst[:, 0:h], src[:, 0:h])
        nc.vector.tensor_copy(dst[:, h:ncols], src[:, h:ncols])

    ones_row = const.tile([1, 512], F32, tag="ones")
    nc.vector.memset(ones_row[:], 1.0)

    # ---- weights: AllGather 1/8 row-shards of W^T over NeuronLink ----
    # io["aux"] is flat fp16: [3*64*512 ws shard | BL*2*SP masks]; the ws
    # part is this core's rows 64c..64c+63 of each W^T.
    wsv = io["aux"][0:3 * 64 * 512].rearrange("(w q d) -> w q d", q=64, d=512)
    m2v = io["aux"][3 * 64 * 512:3 * 64 * 512 + BL * 2 * SP].rearrange(
        "(b t s) -> b t s", t=2, s=SP)
    ws_in = dram.tile([3, 64, 512], FP16)
    ws_all = dram.tile([NCORES, 3, 64, 512], FP16)
    nc.gpsimd.dma_start(ws_in[:], wsv)
    nc.gpsimd.collective_compute(
        "AllGather",
        ALU.bypass,
        replica_groups=[list(range(NCORES))],
        ins=[ws_in[:].opt()],
        outs=[ws_all[:].opt()],
    )
    # ws_all[a, w, q, d] = W_w^T[a*64+q, d]; wt layout [p, j, d'] needs
    # row j*128+p = a*64+q  =>  a = 2j + (p>=64), q = p%64
    wts = {}
    brows = {}
    for w, (wname, bname) in enumerate((("wq", "bq"), ("wk", "bk"),
                                        ("wv", "bv"))):
        wt = const.tile([128, DC, 512], FP16, tag=f"wt_{wname}")
        wts[wname] = wt
        # ws_all dims (a=(j two), w, q, d) -> pick (two, w), land [q, j, d]
        ws_v = ws_all[:].rearrange("(j two) w q d -> two w q j d", two=2)
        for two in range(2):
            nc.gpsimd.dma_start(wt[64 * two:64 * (two + 1), :, :],
                                ws_v[two, w])
        if use_bias:
            br = const.tile([1, 512], F32, tag=f"brow_{bname}")
            nc.sync.dma_start(br[:], io[bname][None, :])
            brows[wname] = br

    def load_x(b):
        """Cast-DMA (fp8e3m4 wire -> fp16) the packed row-slab of batch b
        into [128, SC, 512] tiles (row s at partition s%128, chunk s//128).
        The slab holds SQ q-rows then SK k-rows then SK v-rows, exactly.
        Pad rows (beyond the real count) of k/v tiles are memset to zero:
        stale SBUF contents there could be NaN/Inf, which would poison the
        contraction dims of the scores/AV matmuls.  q pad rows only ever
        reach dropped output columns, so they stay uninitialized."""
        slab = io["x3a"] if b == 0 else io["x3b"]
        xn = {}
        for xname, off, rows, clear in (("xq", 0, SQ, False),
                                        ("xk", SQ, SK, True),
                                        ("xv", SQ + SK, SK, True)):
            t = xstage.tile([128, SC, D], FP16, tag="xn")
            nf, rp = rows // 128, rows % 128
            if clear and rows < SP:
                # pre-zero so the pad region is never stale; the row DMAs
                # below overwrite the real part (engines need 32-aligned
                # partition bases, so a tail-only memset isn't expressible)
                nc.vector.memset(t[:, nf:SC, :], 0.0)
            if nf:
                nc.gpsimd.dma_start(
                    t[:, 0:nf, :],
                    slab[0, off:off + nf * 128].rearrange(
                        "(c p) d -> p c d", p=128),
                )
            if rp:
                nc.gpsimd.dma_start(t[0:rp, nf, :],
                                    slab[0, off + nf * 128:off + rows])
            xn[xname] = t
        return xn

    xn_cur = load_x(0)

    for b in range(BL):
        # ---- per-batch masks ----
        # column layout [128, SC]: element (p, c) = mask[b, c*128 + p]
        qm_t = smalls.tile([128, SC], F32, tag="qm")
        km_t = smalls.tile([128, SC], F32, tag="km")
        with nc.allow_non_contiguous_dma("tiny mask gather"):
            nc.gpsimd.dma_start(qm_t[:], m2v[b, 0].rearrange("(c p) -> p c", p=128))
            nc.gpsimd.dma_start(km_t[:], m2v[b, 1].rearrange("(c p) -> p c", p=128))
        km08 = smalls.tile([128, SC], F32, tag="km08")
        km02 = smalls.tile([128, SC], F32, tag="km02")
        nc.vector.tensor_scalar_mul(km08[:], km_t[:], 0.8)
        nc.vector.tensor_scalar_mul(km02[:], km_t[:], 0.2)

        # ---- transpose prefetched X to XT [128, DC, SP] per input ----
        xts = {}
        for xname in ("xq", "xk", "xv"):
            xt = xtpool.tile([128, DC, SP], FP16, tag=f"xt_{xname}")
            xts[xname] = xt
            for j in range(DC):
                psf = pa.tile([128, PW], FP16, tag="pa")
                ps = psf[:, 0:SP]
                for c in range(SC):
                    nc.tensor.transpose(
                        ps[:, c * 128:(c + 1) * 128],
                        xn_cur[xname][:, c, j * 128:(j + 1) * 128],
                        identb[:],
                    )
                split_copy(xt[:, j, :], ps, SP)

        # ---- projections ----
        # qT/kT: [128, DC, SP]; qT[p, m, s] = q[b, s, m*128+p]
        qt = qkv.tile([128, DC, SP], FP16, tag="qt")
        kt = qkv.tile([128, DC, SP], FP16, tag="kt")
        for proj, wname, dst in (("q", "wq", qt), ("k", "wk", kt)):
            wt = wts[wname]
            xt = xts["xq" if proj == "q" else "xk"]
            for m in range(DC):
                psf = pa.tile([128, PW], F32, tag="pa")
                ps = psf[:, 0:SP]
                for (cs, cw) in chunks:
                    reg = ps[:, cs:cs + cw]
                    for j in range(DC):
                        nc.tensor.matmul(
                            reg,
                            lhsT=wt[:, j, m * 128:(m + 1) * 128],
                            rhs=xt[:, j, cs:cs + cw],
                            start=(j == 0),
                            stop=(j == DC - 1) and not use_bias,
                        )
                    if use_bias:
                        nc.tensor.matmul(
                            reg,
                            lhsT=brows[wname][:, m * 128:(m + 1) * 128],
                            rhs=ones_row[:, 0:cw],
                            start=False,
                            stop=True,
                        )
                # leaky(x) = 0.2*x + relu(0.8*x), chunked so the ACT relu and
                # DVE combine pipeline against the matmul fill
                for (cs, cw) in chunks:
                    sl = slice(cs, cs + cw)
                    r = sepool.tile([128, 512], F32, tag="t02")
                    nc.scalar.activation(r[:, 0:cw], ps[:, sl], AF.Relu,
                                         bias=0.0, scale=0.8)
                    nc.vector.scalar_tensor_tensor(
                        dst[:, m, sl], ps[:, sl], 0.2, r[:, 0:cw],
                        ALU.mult, ALU.add
                    )

        # v_aug: [128, SC, H*65]; per s-chunk c, head h:
        #   cols h*65 .. h*65+63 : leaky(v)[s, h*64+d] * km[s]
        #   col  h*65+64         : km[s]
        vag = qkv.tile([128, SC, H * 65], FP16, tag="vag")
        for c in range(SC):
            psf = pa.tile([128, PW], F32, tag="pa")
            ps = psf[:, 0:512]
            for j in range(DC):
                nc.tensor.matmul(
                    ps[:],
                    lhsT=xts["xv"][:, j, c * 128:(c + 1) * 128],
                    rhs=wts["wv"][:, j, :],
                    start=(j == 0),
                    stop=(j == DC - 1) and not use_bias,
                )
            if use_bias:
                nc.tensor.matmul(
                    ps[:],
                    lhsT=ones_row[:, 0:128],
                    rhs=brows["wv"][:],
                    start=False,
                    stop=True,
                )
            va = vag[:, c, :].rearrange("p (h e) -> p h e", e=65)
            rv = sepool.tile([128, 512], F32, tag="t02")
            nc.scalar.activation(rv[:], ps[:], AF.Relu,
                                 bias=0.0, scale=km08[:, c:c + 1])
            nc.vector.scalar_tensor_tensor(
                va[:, :, 0:64],
                ps[:].rearrange("p (h d) -> p h d", d=64),
                km02[:, c:c + 1],
                rv[:].rearrange("p (h d) -> p h d", d=64),
                ALU.mult,
                ALU.add,
            )
            nc.vector.tensor_copy(
                va[:, :, 64], km_t[:, c:c + 1].to_broadcast((128, H))
            )

        # ---- attention ----
        outs = outsp.tile([128, SC, D], FP16, tag="outs")
        for h in range(H):
            if h == 1 and b + 1 < BL:
                # prefetch next batch's inputs while attention runs; xn slots
                # are free again (this batch's transposes are done)
                xn_cur = load_x(b + 1)
            m = h // 2
            po = 64 * (h % 2)
            pbtf = pb.tile([128, ngrp * 512], F32, tag="pb")
            pbt = pbtf[:, 0:SP]
            for kc in range(SC):
                psf = pa.tile([128, PW], F32, tag="pa")
                ps = psf[:, 0:SP]
                for (cs, cw) in chunks:
                    nc.tensor.matmul(
                        ps[:, cs:cs + cw],
                        lhsT=kt[po:po + 64, m, kc * 128:(kc + 1) * 128],
                        rhs=qt[po:po + 64, m, cs:cs + cw],
                        start=True,
                        stop=True,
                    )
                se = sepool.tile([128, SP], FP16, tag="se")
                nc.scalar.activation(se[:], ps[:], AF.Exp, bias=0.0, scale=0.125)
                for (cs, cw) in chunks:
                    nc.tensor.matmul(
                        pbt[0:65, cs:cs + cw],
                        lhsT=vag[:, kc, h * 65:h * 65 + 65],
                        rhs=se[:, cs:cs + cw],
                        start=(kc == 0),
                        stop=(kc == SC - 1),
                    )
            # outT [65, SP] -> sbuf, transpose back per q-chunk, normalize
            ot = otpool.tile([65, SP], F32, tag="ot")
            nc.vector.tensor_copy(ot[:], pbt[0:65, :])
            pt = pb.tile([128, ngrp * 512], F32, tag="pb")
            for qc in range(SC):
                off = (qc // 4) * 512 + (qc % 4) * 65
                nc.tensor.transpose(
                    pt[:, off:off + 65],
                    ot[:, qc * 128:(qc + 1) * 128],
                    ident[0:65, 0:65],
                )
            rc0 = smalls.tile([128, SC], F32, tag="rc0")
            rc = smalls.tile([128, SC], F32, tag="rc")
            for g in range(ngrp):
                nq = min(SC, g * 4 + 4) - g * 4
                blk = pt[:, g * 512:g * 512 + 65 * nq].rearrange(
                    "p (q e) -> p q e", e=65
                )
                nc.vector.reciprocal(rc0[:, g * 4:g * 4 + nq], blk[:, :, 64])
            nc.vector.tensor_mul(rc[:], rc0[:], qm_t[:])
            for g in range(ngrp):
                nq = min(SC, g * 4 + 4) - g * 4
                blk = pt[:, g * 512:g * 512 + 65 * nq].rearrange(
                    "p (q e) -> p q e", e=65
                )
                nc.vector.tensor_mul(
                    outs[:, g * 4:g * 4 + nq, h * 64:(h + 1) * 64],
                    blk[:, :, 0:64],
                    rc[:, g * 4:g * 4 + nq].unsqueeze(-1).to_broadcast(
                        (128, nq, 64)
                    ),
                )

        # strided store of the SQ real+pad q rows (SWDGE ring, off the
        # load path); rows beyond SQ never reach the wire
        nf, rp = SQ // 128, SQ % 128
        if nf:
            nc.gpsimd.dma_start(
                io["out"][b, 0:nf * 128].rearrange("(c p) d -> p c d", p=128),
                outs[:, 0:nf, :],
            )
        if rp:
            nc.gpsimd.dma_start(io["out"][b, nf * 128:SQ], outs[0:rp, nf, :])


def build_module(use_bias: bool, SP: int, SQ: int, SK: int):
    nc = bacc.Bacc("TRN2", target_bir_lowering=False, debug=False,
                   num_devices=NCORES)
    RT = SQ + 2 * SK
    io = {
        "x3a": nc.dram_tensor("x3a", [1, RT, D], F8E3, kind="ExternalInput").ap(),
        "x3b": nc.dram_tensor("x3b", [1, RT, D], F8E3, kind="ExternalInput").ap(),
        "aux": nc.dram_tensor("aux", [3 * 64 * 512 + BL * 2 * SP], FP16,
                              kind="ExternalInput").ap(),
        "out": nc.dram_tensor("out", [BL, SQ, D], FP16, kind="ExternalOutput").ap(),
    }
    if use_bias:
        for bn in ("bq", "bk", "bv"):
            io[bn] = nc.dram_tensor(bn, [D], F32, kind="ExternalInput").ap()
    with tile.TileContext(nc) as tc:
        with ExitStack() as ctx:
            _mha_body(ctx, tc, io, use_bias, SP, SQ, SK)
    nc.compile()
    return nc


_REPLICATED = {"bq", "bk", "bv"}

_CACHE = {}


def _build_state(use_bias: bool, SP: int, SQ: int, SK: int):
    nc = build_module(use_bias, SP, SQ, SK)
    install_neuronx_cc_hook()

    partition_name = (
        nc.partition_id_tensor.name if nc.partition_id_tensor else None
    )
    in_names, out_names, out_avals, in_meta = [], [], [], {}
    for alloc in nc.m.functions[0].allocations:
        if not isinstance(alloc, mybir.MemoryLocationSet):
            continue
        name = alloc.memorylocations[0].name
        if alloc.kind == "ExternalInput":
            if name != partition_name:
                in_names.append(name)
                in_meta[name] = (tuple(alloc.tensor_shape),
                                 mybir.dt.np(alloc.dtype))
        elif alloc.kind == "ExternalOutput":
            out_names.append(name)
            out_avals.append(jax.core.ShapedArray(
                tuple(alloc.tensor_shape), mybir.dt.np(alloc.dtype)))
    n_params = len(in_names)
    n_outs = len(out_names)
    all_names = in_names + out_names
    if partition_name is not None:
        all_names.append(partition_name)
    donate = tuple(range(n_params, n_params + n_outs))

    def _body(*args):
        operands = list(args)
        if partition_name is not None:
            operands.append(partition_id_tensor())
        return tuple(_bass_exec_p.bind(
            *operands,
            out_avals=tuple(out_avals),
            in_names=tuple(all_names),
            out_names=tuple(out_names),
            lowering_input_output_aliases=(),
            sim_require_finite=True,
            sim_require_nnan=True,
            nc=nc,
        ))

    devices = jax.devices()[:NCORES]
    mesh = Mesh(np.asarray(devices), ("core",))
    sh_split = NamedSharding(mesh, PartitionSpec("core"))
    sh_rep = NamedSharding(mesh, PartitionSpec())

    in_specs, arg_specs = [], []
    for name in in_names:
        shape, dt = in_meta[name]
        if name in _REPLICATED:
            in_specs.append(PartitionSpec())
            arg_specs.append(jax.ShapeDtypeStruct(shape, dt, sharding=sh_rep))
        else:
            in_specs.append(PartitionSpec("core"))
            arg_specs.append(jax.ShapeDtypeStruct(
                (NCORES * shape[0],) + shape[1:], dt, sharding=sh_split))
    for i in range(n_outs):
        in_specs.append(PartitionSpec("core"))
        shp = out_avals[i].shape
        arg_specs.append(jax.ShapeDtypeStruct(
            (NCORES * shp[0],) + shp[1:], out_avals[i].dtype,
            sharding=sh_split))
    out_specs = (PartitionSpec("core"),) * n_outs

    sharded = jax.jit(
        jax.shard_map(_body, mesh=mesh, in_specs=tuple(in_specs),
                      out_specs=out_specs, check_vma=False),
        donate_argnums=donate,
        keep_unused=True,
    )
    compiled = sharded.lower(*arg_specs).compile()

    out_shape = out_avals[0].shape
    zeros_fn = jax.jit(
        lambda: jnp.zeros((NCORES * out_shape[0],) + out_shape[1:],
                          out_avals[0].dtype),
        out_shardings=sh_split,
    )
    jax.block_until_ready(zeros_fn())

    return {
        "compiled": compiled,
        "zeros_fn": zeros_fn,
        "in_names": in_names,
        "sh_split": sh_split,
        "sh_rep": sh_rep,
    }


def _get_state(use_bias: bool, SP: int, SQ: int, SK: int):
    key = (use_bias, SP, SQ, SK)
    if key not in _CACHE:
        _CACHE[key] = _build_state(use_bias, SP, SQ, SK)
    return _CACHE[key]


def kernel(query, key, value, q_mask, k_mask, WQ, bQ, WK, bK, WV, bV):
    use_bias = bool(np.any(bQ) or np.any(bK) or np.any(bV))
    qm = np.asarray(q_mask)
    km = np.asarray(k_mask)
    qnz = [np.flatnonzero(qm[b]) for b in range(B)]
    knz = [np.flatnonzero(km[b]) for b in range(B)]
    SQ = max(max((len(i) for i in qnz), default=0), 1)
    SK = max(max((len(i) for i in knz), default=0), 1)
    SP = min(S, ((max(SQ, SK) + 127) // 128) * 128)

    st = _get_state(use_bias, SP, SQ, SK)
    put = jax.device_put
    sh_split, sh_rep = st["sh_split"], st["sh_rep"]

    # donated zero output buffer is built on device: no wire traffic
    zeros = st["zeros_fn"]()

    # aux: per-core [ws shard (rows 64c..64c+63 of each W^T) | masks]
    WSE = 3 * 64 * 512
    aux = np.zeros((NCORES, WSE + BL * 2 * SP), NP_F16)
    wsa = aux[:, :WSE].reshape(NCORES, 3, 64, 512)
    for w, W in enumerate((WQ, WK, WV)):
        wsa[:, w] = np.asarray(W).T.astype(NP_F16).reshape(NCORES, 64, 512)
    m2a = aux[:, WSE:].reshape(NCORES, BL, 2, SP)
    for b in range(B):
        m2a[b // BL, b % BL, 0, :len(qnz[b])] = 1.0
        m2a[b // BL, b % BL, 1, :len(knz[b])] = 1.0
    dev = {"aux": put(aux.reshape(-1), sh_split)}

    # pack mask-selected rows into two fp8e3m4 slabs (each core's batch 0
    # in x3a, batch 1 in x3b).  Slab rows: SQ q-rows, SK k-rows, SK v-rows.
    # Slabs must be zero-filled: wire-pad rows of k/v (between a batch's
    # real count and SK) reach the AV contraction dim, where stale NaN
    # bytes would poison every output even with the mask gate (0*NaN=NaN).
    # Cast via the f16 LUT, gathering and casting into reused scratch.
    lut = _f16_to_e3m4_lut()
    RT = SQ + 2 * SK
    gbuf = np.empty((max(SQ, SK), D), np.float32)
    hbuf = np.empty((max(SQ, SK), D), np.float16)
    srcs = ((np.asarray(query), qnz, 0, SQ), (np.asarray(key), knz, SQ, SK),
            (np.asarray(value), knz, SQ + SK, SK))
    for name, par in (("x3a", 0), ("x3b", 1)):
        slab = np.zeros((NCORES, RT, D), np.uint8)
        for c in range(NCORES):
            b = 2 * c + par
            for x, nz, off, _rows in srcs:
                idx = nz[b]
                n = len(idx)
                np.take(x[b], idx, axis=0, out=gbuf[:n])
                np.copyto(hbuf[:n], gbuf[:n], casting="unsafe")
                np.take(lut, hbuf[:n].view(np.uint16),
                        out=slab[c, off:off + n])
        dev[name] = put(slab.view(NP_F8E3), sh_split)
    if use_bias:
        dev["bq"] = put(np.asarray(bQ, np.float32), sh_rep)
        dev["bk"] = put(np.asarray(bK, np.float32), sh_rep)
        dev["bv"] = put(np.asarray(bV, np.float32), sh_rep)

    args = [dev[name] for name in st["in_names"]]
    (out_dev,) = st["compiled"](*args, zeros)
    outp = np.asarray(out_dev)

    out = np.zeros((B, S, D), np.float32)
    for b in range(B):
        idx = qnz[b]
        out[b, idx] = outp[b, :len(idx)].astype(np.float32)
    return out
